# revision 42
# baseline (speedup 1.0000x reference)
"""GQA attention kernel for 8 TRN2 NeuronCores (axon PJRT path).

The wall-clock of a call is dominated by host<->device transfer over the
axon tunnel (~60 MB/s, serialized on this box's single CPU), so the design
minimizes wire bytes and host CPU passes:

- Sharding: core c = (batch b = c//4, kv-head h = c%4).
- Activations ship as disjoint per-core slices (batch b's seq rows
  [h*512:(h+1)*512], natural layout) in a 12-bit float format (fp16 with
  the low 4 mantissa bits rounded away), split into a hi-byte plane and a
  row-paired nibble plane. Weights ship the same way, additionally halved
  per batch. On device: integer unpack to fp16, PE-transpose, then an
  AllGather over the 4-core batch group (NeuronLink) rebuilds the full
  transposed activations; pair AllGathers rebuild the full weights.
- Per-core attention (4 query heads of one KV group) runs as in the
  baseline: causal S^T layout, softmax reduction folded into the PV matmul
  via an appended ones-column on V, f32r tensor ops. Projections consume
  fp16 operands directly (fp32 PSUM accumulation).
- The per-core partial output (its 256 columns of Wo) is summed across the
  group with an in-kernel fp32 ReduceScatter; each core packs its disjoint
  [512, 1024] slice back to the 12-bit wire format and the host
  reassembles with contiguous byte-plane writes.
- RoPE tables, masks, identity/rotation matrices and the output zero
  buffers are cached on device once; the jitted shard_map callable is
  cached too. A warm call ships ~22.8 MB in and ~6.3 MB out.
"""
import sys, os
sys.path.insert(0, "/opt/trn_rl_repo")
os.environ.setdefault("MYCRO_LOCAL_CACHE", "1")

import numpy as np
from concurrent.futures import ThreadPoolExecutor
from contextlib import ExitStack

import concourse.bass as bass
import concourse.tile as tile
from concourse import bacc, mybir
import jax
from jax.sharding import Mesh, PartitionSpec, NamedSharding
from jax.experimental.shard_map import shard_map
from concourse.bass2jax import (
    _bass_exec_p,
    install_neuronx_cc_hook,
    partition_id_tensor,
)

F32, F32R, FP16 = mybir.dt.float32, mybir.dt.float32r, mybir.dt.float16
U8, U16 = mybir.dt.uint8, mybir.dt.uint16
AF = mybir.ActivationFunctionType
ALU = mybir.AluOpType

B, S, DM = 2, 2048, 1024
H, HKV, DK = 16, 4, 64
G = H // HKV                 # 4 query heads per core
NKT = DM // 128              # 8 dmodel k-tiles
NSQ = S // 512               # 4 sq tiles
NSK = S // 128               # 16 sk tiles
N_CORES = 8
GROUPS = [[0, 1, 2, 3], [4, 5, 6, 7]]
PAIRS = [[0, 4], [1, 5], [2, 6], [3, 7]]   # same kv-head, other batch
SLOC = S // G                # 512: seq rows shipped per core
XROWS = 3 * DM               # 3072: q|k|v transposed rows per core slice

_runtime = None


def _build():
    nc = bacc.Bacc("TRN2", target_bir_lowering=False, debug=False,
                   num_devices=N_CORES)
    inp = {}
    WCOL = G * DK + 2 * DK                 # 384
    for name, shape, dt in [
        # 12-bit wire format: per fp16 value v, code = RN((bits(v)) >> 4);
        # hi byte = code >> 4; low nibbles pair row r with row r+64 of each
        # 128-row block. x ships in natural layout [local seq, q|k|v
        # dmodel]; the device transposes after unpacking
        ("xhi", [SLOC, XROWS], U8),
        ("xnib", [SLOC // 2, XROWS], U8),
        # batch-half of the packed weights [wqT | wkT | wvT]; wq pre-scaled
        # by 0.125; the pair AllGather with the same-head core of the other
        # batch rebuilds the full [DM, 384] block
        ("whqhi", [DM // 2, WCOL], U8),
        ("whqnib", [DM // 4, WCOL], U8),
        ("whohi", [G * DK // 2, DM], U8),  # batch-half of woT
        ("whonib", [G * DK // 4, DM], U8),
        ("cos2", [128, S], F32),
        ("sin2", [128, S], F32),
        ("r2T", [128, 128], F32),
        ("ident", [64, 64], F32),
        ("ident128", [128, 128], FP16),
        ("masks", [128, 4 * 512], F32),
    ]:
        inp[name] = nc.dram_tensor(name, shape, dt, kind="ExternalInput").ap()
    # output in the same 12-bit wire format: rows 0:SLOC hi bytes, rows
    # SLOC:SLOC*3//2 the row-paired nibbles
    out = nc.dram_tensor("out", [SLOC + SLOC // 2, DM], U8,
                         kind="ExternalOutput").ap()

    xsT_b = nc.dram_tensor("xsT_b", [XROWS, SLOC], FP16, kind="Internal").ap()
    xgT = nc.dram_tensor("xgT", [G * XROWS, SLOC], FP16, kind="Internal").ap()
    whq_b = nc.dram_tensor("whq_b", [DM // 2, WCOL], FP16, kind="Internal").ap()
    whq_g = nc.dram_tensor("whq_g", [DM, WCOL], FP16, kind="Internal").ap()
    who_b = nc.dram_tensor("who_b", [G * DK // 2, DM], FP16, kind="Internal").ap()
    who_g = nc.dram_tensor("who_g", [G * DK, DM], FP16, kind="Internal").ap()
    partial = nc.dram_tensor("partial", [S, DM], F32, kind="Internal").ap()
    rsout = nc.dram_tensor("rsout", [SLOC, DM], F32, kind="Internal").ap()

    with tile.TileContext(nc) as tc, ExitStack() as ctx:
        const = ctx.enter_context(tc.tile_pool(name="const", bufs=1))
        sb = ctx.enter_context(tc.tile_pool(name="sb", bufs=2))
        sbx = ctx.enter_context(tc.tile_pool(name="sbx", bufs=8))
        ps = ctx.enter_context(tc.tile_pool(name="ps", bufs=2, space="PSUM"))
        ps_acc = ctx.enter_context(tc.tile_pool(name="ps_acc", bufs=2, space="PSUM"))
        ps_tr = ctx.enter_context(tc.tile_pool(name="ps_tr", bufs=2, space="PSUM"))
        ps_tx = ctx.enter_context(tc.tile_pool(name="ps_tx", bufs=2, space="PSUM"))

        up = ctx.enter_context(tc.tile_pool(name="up", bufs=3))

        id128 = const.tile([128, 128], FP16, tag="id128")
        nc.gpsimd.dma_start(id128[:], inp["ident128"][:])

        # unpack a [128, w] block of the 12-bit wire format into an fp16
        # SBUF tile: bits16 = (hi << 8) | (nibble << 4); the [64, w] nibble
        # block is replicated into both partition halves, then masked per
        # half (high nibble -> rows 0:64, low nibble -> rows 64:128)
        def unpack_block(hi_ap, nib_ap, w):
            h8 = up.tile([128, w], U8, tag="h8")
            nc.sync.dma_start(h8[:], hi_ap)
            n8 = up.tile([128, w], U8, tag="n8")
            nc.sync.dma_start(n8[0:64, :], nib_ap)
            nc.sync.dma_start(n8[64:128, :], nib_ap)
            bits = up.tile([128, w], U16, tag="bits")
            nc.vector.tensor_copy(bits[:], h8[:])
            nc.vector.tensor_scalar(bits[:], bits[:], 8, None,
                                    op0=ALU.logical_shift_left)
            n16 = up.tile([128, w], U16, tag="n16")
            nc.vector.tensor_copy(n16[:], n8[:])
            nc.vector.tensor_scalar(n16[0:64, :], n16[0:64, :], 0xF0, None,
                                    op0=ALU.bitwise_and)
            nc.vector.tensor_scalar(n16[64:128, :], n16[64:128, :], 4, 0xF0,
                                    op0=ALU.logical_shift_left,
                                    op1=ALU.bitwise_and)
            nc.vector.tensor_tensor(bits[:], bits[:], n16[:],
                                    op=ALU.bitwise_or)
            return bits

        def unpack_to(hi, nib, dst, rows, w):
            for i in range(rows // 128):
                r = slice(i * 128, (i + 1) * 128)
                rn = slice(i * 64, (i + 1) * 64)
                bits = unpack_block(hi[r, :], nib[rn, :], w)
                nc.sync.dma_start(dst[r, :], bits[:].bitcast(FP16))

        # rebuild full weights from the batch-halves (pair = same kv-head,
        # other batch)
        unpack_to(inp["whqhi"], inp["whqnib"], whq_b, DM // 2, WCOL)
        nc.gpsimd.collective_compute(
            "AllGather", mybir.AluOpType.bypass, replica_groups=PAIRS,
            ins=[whq_b[:]], outs=[whq_g[:]],
        )
        unpack_to(inp["whohi"], inp["whonib"], who_b, G * DK // 2, DM)
        nc.gpsimd.collective_compute(
            "AllGather", mybir.AluOpType.bypass, replica_groups=PAIRS,
            ins=[who_b[:]], outs=[who_g[:]],
        )
        # x: unpack natural [128, 3072] blocks, PE-transpose into xsT_b,
        # then gather the full transposed activations for this batch across
        # the 4-core group (member h contributed seq cols [h*512:(h+1)*512])
        for i in range(SLOC // 128):
            ri = slice(i * 128, (i + 1) * 128)
            rn = slice(i * 64, (i + 1) * 64)
            for p in range(3):
                bits = unpack_block(
                    inp["xhi"][ri, p * DM:(p + 1) * DM],
                    inp["xnib"][rn, p * DM:(p + 1) * DM], DM)
                x16 = bits[:].bitcast(FP16)
                for j in range(DM // 128):
                    pst = ps_tx.tile([128, 128], FP16, tag="tx")
                    nc.tensor.transpose(pst[:], x16[:, j * 128:(j + 1) * 128],
                                        id128[:])
                    stg = sb.tile([128, 128], FP16, tag="stg")
                    nc.vector.tensor_copy(stg[:], pst[:])
                    nc.sync.dma_start(
                        xsT_b[p * DM + j * 128:p * DM + (j + 1) * 128,
                              i * 128:(i + 1) * 128],
                        stg[:])
        nc.gpsimd.collective_compute(
            "AllGather", mybir.AluOpType.bypass, replica_groups=GROUPS,
            ins=[xsT_b[:]], outs=[xgT[:]],
        )

        def load_const(name, shape, dtype=F32R):
            if dtype == F32:
                t = const.tile(shape, F32, tag=name + "_raw")
                nc.sync.dma_start(t[:], inp[name][:])
                return t
            r = const.tile(shape, F32R, tag=name)
            nc.gpsimd.dma_start(r[:], inp[name][:])
            return r

        # weights: whq_g [DM, 384] fp16 -> SBUF [128, NKT*M] (k-tiles on free
        # dim); columns 0:256 wq, 256:320 wk, 320:384 wv
        def load_wT(col0, m, tag):
            r = const.tile([128, NKT * m], FP16, tag=tag)
            for kt in range(NKT):
                nc.gpsimd.dma_start(r[:, kt * m:(kt + 1) * m],
                                    whq_g[kt * 128:(kt + 1) * 128,
                                          col0:col0 + m])
            return r

        wq_sb = load_wT(0, G * DK, "wq_sb")           # [128, 8*256]
        wk_sb = load_wT(G * DK, DK, "wk_sb")          # [128, 8*64]
        wv_sb = load_wT(G * DK + DK, DK, "wv_sb")
        wo_sb = const.tile([128, 2 * DM], FP16, tag="wo_sb")
        nc.gpsimd.dma_start(wo_sb[:, 0:DM], who_g[0:128, :])
        nc.gpsimd.dma_start(wo_sb[:, DM:2 * DM], who_g[128:256, :])
        cos_sb = load_const("cos2", [128, S], F32)
        sin_sb = load_const("sin2", [128, S], F32)
        r2_sb = load_const("r2T", [128, 128])
        id_sb = load_const("ident", [64, 64])
        mask_sb = load_const("masks", [128, 4 * 512], F32)

        # persistent activations
        qt = [const.tile([128, S], F32R, tag=f"qt{i}", name=f"qt{i}") for i in range(2)]
        krope = const.tile([64, S], F32R, tag="krope")
        khi = const.tile([128, S], F32R, tag="khi")
        v_sb = const.tile([128, NSK, 65], F32R, tag="v_sb")
        ot = [const.tile([128, S], FP16, tag=f"ot{i}", name=f"ot{i}") for i in range(2)]

        # x chunk [128, 512] fp16 from the gathered transposed activations:
        # member st's block holds global seq cols [st*512:(st+1)*512]
        def x_chunk(part, kt, st):
            r = sbx.tile([128, 512], FP16, tag=f"x{part}_r")
            base = st * XROWS + part * DM + kt * 128
            nc.gpsimd.dma_start(r[:], xgT[base:base + 128, :])
            return r

        # ---- Q projection + rope (heads packed 2+2 into qt[0], qt[1])
        for st in range(NSQ):
            xq = [x_chunk(0, kt, st) for kt in range(NKT)]
            for half in range(2):
                psQ = ps.tile([128, 512], F32, tag="big")
                for kt in range(NKT):
                    o = kt * G * DK + half * 128
                    nc.tensor.matmul(psQ[:], wq_sb[:, o:o + 128], xq[kt][:],
                                     start=(kt == 0), stop=(kt == NKT - 1))
                qsb = sb.tile([128, 512], F32R, tag="pcopy")
                nc.vector.tensor_copy(qsb[:], psQ[:])
                psRot = ps.tile([128, 512], F32, tag="big")
                nc.tensor.matmul(psRot[:], r2_sb[:], qsb[:], start=True, stop=True)
                t1 = sb.tile([128, 512], F32, tag="t1")
                nc.vector.tensor_mul(t1[:], qsb[:], cos_sb[:, st * 512:(st + 1) * 512])
                t2 = sb.tile([128, 512], F32, tag="t2")
                nc.vector.tensor_mul(t2[:], psRot[:], sin_sb[:, st * 512:(st + 1) * 512])
                nc.vector.tensor_add(qt[half][:, st * 512:(st + 1) * 512], t1[:], t2[:])

        # ---- K + V projections
        for st in range(NSQ):
            xk = [x_chunk(1, kt, st) for kt in range(NKT)]
            xv = [x_chunk(2, kt, st) for kt in range(NKT)]
            psK = ps.tile([64, 512], F32, tag="big")
            for kt in range(NKT):
                nc.tensor.matmul(psK[:], wk_sb[:, kt * DK:(kt + 1) * DK], xk[kt][:],
                                 start=(kt == 0), stop=(kt == NKT - 1))
            ksb = sb.tile([64, 512], F32R, tag="pcopy")
            nc.vector.tensor_copy(ksb[:], psK[:])
            psRotK = ps.tile([64, 512], F32, tag="big")
            nc.tensor.matmul(psRotK[:], r2_sb[0:64, 0:64], ksb[:], start=True, stop=True)
            k1 = sb.tile([64, 512], F32, tag="t1")
            nc.vector.tensor_mul(k1[:], ksb[:], cos_sb[0:64, st * 512:(st + 1) * 512])
            k2 = sb.tile([64, 512], F32, tag="t2")
            nc.vector.tensor_mul(k2[:], psRotK[:], sin_sb[0:64, st * 512:(st + 1) * 512])
            nc.vector.tensor_add(krope[:, st * 512:(st + 1) * 512], k1[:], k2[:])
            nc.sync.dma_start(khi[64:128, st * 512:(st + 1) * 512],
                              krope[:, st * 512:(st + 1) * 512])

            psVT = ps.tile([64, 512], F32, tag="big")
            for kt in range(NKT):
                nc.tensor.matmul(psVT[:], wv_sb[:, kt * DK:(kt + 1) * DK], xv[kt][:],
                                 start=(kt == 0), stop=(kt == NKT - 1))
            vtsb = sb.tile([64, 512], F32R, tag="pcopy")
            nc.vector.tensor_copy(vtsb[:], psVT[:])
            for j in range(4):
                psVtr = ps_tr.tile([128, 64], F32R, tag="tr")
                nc.tensor.transpose(psVtr[:], vtsb[:, j * 128:(j + 1) * 128], id_sb[:])
                nc.vector.tensor_copy(v_sb[:, st * 4 + j, 0:64], psVtr[:])
        nc.gpsimd.memset(v_sb[:, :, 64:65].bitcast(F32), 1.0)

        # ---- attention: h in 4 query heads, st in 4 sq tiles (causal sk range)
        for h in range(G):
            half, sub = h // 2, h % 2
            for st in range(NSQ):
                psO = ps_acc.tile([65, 512], F32, tag="acc")
                nsk = 4 * st + 4
                for skt in range(nsk):
                    di = skt - 4 * st            # >=0 on diagonal tiles
                    psS = ps.tile([128, 512], F32, tag="big")
                    if sub == 0:
                        lhsT = krope[:, skt * 128:(skt + 1) * 128]
                        rhs = qt[half][0:64, st * 512:(st + 1) * 512]
                    else:
                        lhsT = khi[64:128, skt * 128:(skt + 1) * 128]
                        rhs = qt[half][64:128, st * 512:(st + 1) * 512]
                    nc.tensor.matmul(psS[:], lhsT, rhs, start=True, stop=True)
                    pt2 = sb.tile([128, 512], F32R, tag="pt2")
                    if di >= 0:
                        pt = sb.tile([128, 512], F32, tag="pt")
                        nc.scalar.activation(pt[:], psS[:], AF.Exp)
                        nc.vector.tensor_mul(pt2[:], pt[:],
                                             mask_sb[:, di * 512:(di + 1) * 512])
                    else:
                        nc.scalar.activation(pt2[:], psS[:], AF.Exp)
                    nc.tensor.matmul(psO[:], v_sb[:, skt, :], pt2[:],
                                     start=(skt == 0), stop=(skt == nsk - 1))
                recip = sb.tile([128, 512], F32, tag="recip")
                nc.vector.reciprocal(recip[64:65, :], psO[64:65, :])
                recip0 = sb.tile([1, 512], F32, tag="recip0")
                nc.sync.dma_start(recip0[:], recip[64:65, :])
                bcast = sb.tile([64, 512], F32, tag="bcast")
                nc.gpsimd.partition_broadcast(bcast[:], recip0[:])
                if sub == 0:
                    nc.vector.tensor_mul(ot[half][0:64, st * 512:(st + 1) * 512],
                                         psO[0:64, :], bcast[:])
                else:
                    tmp = sb.tile([64, 512], FP16, tag="otmp")
                    nc.vector.tensor_mul(tmp[:], psO[0:64, :], bcast[:])
                    nc.sync.dma_start(ot[half][64:128, st * 512:(st + 1) * 512], tmp[:])

        # ---- output projection into the fp32 partial buffer
        for st in range(S // 128):
            for dt in range(2):
                psF = ps.tile([128, 512], F32, tag="big")
                nc.tensor.matmul(psF[:], ot[0][:, st * 128:(st + 1) * 128],
                                 wo_sb[:, dt * 512:(dt + 1) * 512],
                                 start=True, stop=False)
                nc.tensor.matmul(psF[:], ot[1][:, st * 128:(st + 1) * 128],
                                 wo_sb[:, DM + dt * 512:DM + (dt + 1) * 512],
                                 start=False, stop=True)
                osb = sb.tile([128, 512], F32, tag="osb")
                nc.scalar.copy(osb[:], psF[:])
                nc.sync.dma_start(partial[st * 128:(st + 1) * 128,
                                          dt * 512:(dt + 1) * 512], osb[:])

        # ---- sum partials across the group; member h keeps seq rows
        # [h*512:(h+1)*512]; emit as fp16
        nc.gpsimd.collective_compute(
            "ReduceScatter", mybir.AluOpType.add, replica_groups=GROUPS,
            ins=[partial[:]], outs=[rsout[:]],
        )
        for i in range(SLOC // 128):
            rs_sb = sb.tile([128, DM], F32, tag="rs_sb")
            nc.sync.dma_start(rs_sb[:], rsout[i * 128:(i + 1) * 128, :])
            rs16 = sb.tile([128, DM], FP16, tag="rs16")
            nc.scalar.copy(rs16[:], rs_sb[:])
            # 12-bit pack: t = bits+8; hi = t>>8; nib row r = (t[r] & 0xF0)
            # | ((t[r+64] >> 4) & 0xF)
            t = sb.tile([128, DM], U16, tag="pk_t")
            nc.vector.tensor_scalar(t[:], rs16[:].bitcast(U16), 8, None,
                                    op0=ALU.add)
            hs = sb.tile([128, DM], U16, tag="pk_h")
            nc.vector.tensor_scalar(hs[:], t[:], 8, None,
                                    op0=ALU.logical_shift_right)
            h8 = sb.tile([128, DM], U8, tag="pk_h8")
            nc.vector.tensor_copy(h8[:], hs[:])
            tlo = sb.tile([64, DM], U16, tag="pk_tlo")
            nc.sync.dma_start(tlo[:], t[64:128, :])   # partition shift
            nb = sb.tile([64, DM], U16, tag="pk_nb")
            nc.vector.tensor_scalar(nb[:], tlo[:], 4, 0xF,
                                    op0=ALU.logical_shift_right,
                                    op1=ALU.bitwise_and)
            ev = sb.tile([64, DM], U16, tag="pk_ev")
            nc.vector.tensor_scalar(ev[:], t[0:64, :], 0xF0, None,
                                    op0=ALU.bitwise_and)
            nc.vector.tensor_tensor(nb[:], nb[:], ev[:], op=ALU.bitwise_or)
            n8 = sb.tile([64, DM], U8, tag="pk_n8")
            nc.vector.tensor_copy(n8[:], nb[:])
            nc.sync.dma_start(out[i * 128:(i + 1) * 128, :], h8[:])
            nc.sync.dma_start(out[SLOC + i * 64:SLOC + (i + 1) * 64, :],
                              n8[:])

    nc.compile()
    return nc


def _make_consts():
    inv_freq = 1.0 / (10000.0 ** (np.arange(0, DK, 2, dtype=np.float64) / DK))
    t = np.arange(S, dtype=np.float64)
    freqs = np.einsum("s,f->sf", t, inv_freq)
    emb = np.concatenate([freqs, freqs], axis=-1)
    cos = np.cos(emb).astype(np.float32).T.copy()   # [64, S]
    sin = np.sin(emb).astype(np.float32).T.copy()
    cos2 = np.concatenate([cos, cos], axis=0).copy()
    sin2 = np.concatenate([sin, sin], axis=0).copy()
    R = np.zeros((DK, DK), np.float32)
    half = DK // 2
    for d in range(half):
        R[d, d + half] = -1.0
        R[d + half, d] = 1.0
    r2T = np.zeros((128, 128), np.float32)
    r2T[0:64, 0:64] = R.T
    r2T[64:128, 64:128] = R.T
    ident = np.eye(64, dtype=np.float32)
    ident128 = np.eye(128, dtype=np.float16)
    masks = np.zeros((128, 4 * 512), np.float32)
    rr = np.arange(128)[:, None]
    cc = np.arange(512)[None, :]
    for i in range(4):
        masks[:, i * 512:(i + 1) * 512] = (rr <= cc - 128 * i).astype(np.float32)
    return {"cos2": cos2, "sin2": sin2, "r2T": r2T, "ident": ident,
            "ident128": ident128, "masks": masks}


def _init_runtime():
    nc = _build()
    install_neuronx_cc_hook()
    partition_name = nc.partition_id_tensor.name if nc.partition_id_tensor else None
    in_names, out_names, out_avals = [], [], []
    for alloc in nc.m.functions[0].allocations:
        if not isinstance(alloc, mybir.MemoryLocationSet):
            continue
        name = alloc.memorylocations[0].name
        if alloc.kind == "ExternalInput":
            if name != partition_name:
                in_names.append(name)
        elif alloc.kind == "ExternalOutput":
            out_names.append(name)
            out_avals.append(jax.core.ShapedArray(
                tuple(alloc.tensor_shape), mybir.dt.np(alloc.dtype)))
    all_in = list(in_names) + list(out_names)
    if partition_name is not None:
        all_in.append(partition_name)

    def _body(*args):
        operands = list(args)
        if partition_name is not None:
            operands.append(partition_id_tensor())
        return tuple(_bass_exec_p.bind(
            *operands, out_avals=tuple(out_avals), in_names=tuple(all_in),
            out_names=tuple(out_names), lowering_input_output_aliases=(),
            sim_require_finite=True, sim_require_nnan=True, nc=nc))

    mesh = Mesh(np.asarray(jax.devices()[:N_CORES]), ("core",))
    nspec = len(in_names) + len(out_names)
    fn = jax.jit(shard_map(_body, mesh=mesh,
                           in_specs=(PartitionSpec("core"),) * nspec,
                           out_specs=(PartitionSpec("core"),) * len(out_names),
                           check_rep=False))
    sh = NamedSharding(mesh, PartitionSpec("core"))

    consts = _make_consts()
    dev_consts = {
        name: jax.device_put(np.tile(arr, (N_CORES,) + (1,) * (arr.ndim - 1)), sh)
        for name, arr in consts.items()
    }
    dev_zeros = [
        jax.device_put(np.zeros((N_CORES * a.shape[0], *a.shape[1:]), a.dtype), sh)
        for a in out_avals
    ]
    return {
        "fn": fn, "sh": sh, "in_names": in_names,
        "dev_consts": dev_consts, "dev_zeros": dev_zeros,
        "post_bits": np.empty((N_CORES * SLOC, DM), np.uint16),
        "post_nib": np.empty((N_CORES * SLOC // 2 // 64, 64, DM), np.uint8),
    }


_pk_scratch = {}


def _pack12_into(x16, hi_out, nib_out):
    """fp16 [R, W] -> 12-bit code RN(bits >> 4) into hi_out [R, W] u8 and
    nib_out [R//2, W] u8. The +8 round carry propagates correctly through
    IEEE bit patterns; values here are far from the fp16 overflow region.
    Nibbles pair row r with row r+64 within each 128-row block, so every
    pass is a contiguous SIMD op into preallocated scratch (the box has a
    single CPU, and host cycles trade 1:1 against wire time); the device
    applies them as partition-half ops."""
    R, W = x16.shape
    sc = _pk_scratch.get((R, W))
    if sc is None:
        sc = (np.empty((R, W), np.uint16),
              np.empty((R // 128, 64, W), np.uint16),
              np.empty((R // 128, 64, W), np.uint16))
        _pk_scratch[(R, W)] = sc
    t, na, nb = sc
    np.add(x16.view(np.uint16), np.uint16(8), out=t)
    hi_out[:] = t.view(np.uint8)[:, 1::2]
    tb = t.reshape(-1, 128, W)
    np.bitwise_and(tb[:, 0:64], np.uint16(0xF0), out=na)
    np.right_shift(tb[:, 64:128], np.uint16(4), out=nb)
    np.bitwise_and(nb, np.uint16(0xF), out=nb)
    np.bitwise_or(na, nb, out=na)
    nib_out[:] = na.view(np.uint8).reshape(R // 2, 2 * W)[:, 0::2]


def _pack12(x16):
    hi = np.empty(x16.shape, np.uint8)
    nib = np.empty((x16.shape[0] // 2, x16.shape[1]), np.uint8)
    _pack12_into(x16, hi, nib)
    return hi, nib


def _host_xsT(query, key, value):
    """Natural-layout packed slices; the device transposes (PE) after
    unpacking, so the host never does a strided fp16 transpose."""
    hi = np.empty((N_CORES * SLOC, XROWS), np.uint8)
    nib = np.empty((N_CORES * SLOC // 2, XROWS), np.uint8)
    buf = np.empty((SLOC, XROWS), np.float16)
    for c in range(N_CORES):
        b, h = c // HKV, c % HKV
        sl = slice(h * SLOC, (h + 1) * SLOC)
        buf[:, 0:DM] = query[b, sl, :]                 # casts on assignment
        buf[:, DM:2 * DM] = key[b, sl, :]
        buf[:, 2 * DM:3 * DM] = value[b, sl, :]
        _pack12_into(buf, hi[c * SLOC:(c + 1) * SLOC],
                     nib[c * SLOC // 2:(c + 1) * SLOC // 2])
    return hi, nib


def _host_weights(Wq, Wk, Wv, Wo):
    HD = DM // 2
    WCOL = G * DK + 2 * DK
    whq_g = np.empty((N_CORES * HD, WCOL), np.float16)
    who_g = np.empty((N_CORES * G * DK // 2, DM), np.float16)
    for c in range(N_CORES):
        b, h = c // HKV, c % HKV
        r = slice(b * HD, (b + 1) * HD)            # this core's dmodel half
        blk = whq_g[c * HD:(c + 1) * HD]
        blk[:, 0:G * DK] = Wq[h * G * DK:(h + 1) * G * DK, :].T[r] * np.float32(0.125)
        blk[:, G * DK:G * DK + DK] = Wk[h * DK:(h + 1) * DK, :].T[r]
        blk[:, G * DK + DK:WCOL] = Wv[h * DK:(h + 1) * DK, :].T[r]
        ro = slice(b * G * DK // 2, (b + 1) * G * DK // 2)
        who_g[c * G * DK // 2:(c + 1) * G * DK // 2] = \
            Wo[:, h * G * DK:(h + 1) * G * DK].T[ro]
    return _pack12(whq_g) + _pack12(who_g)


def kernel(query, key, value, Wq, Wk, Wv, Wo):
    global _runtime
    query, key, value = (np.asarray(a, np.float32) for a in (query, key, value))
    Wq, Wk, Wv, Wo = (np.asarray(a, np.float32) for a in (Wq, Wk, Wv, Wo))
    if _runtime is None:
        _runtime = _init_runtime()
    rt = _runtime
    # device_put is async: stream the quick-to-build weight arrays first so
    # the big activation array's host build overlaps their wire time
    w4 = _host_weights(Wq, Wk, Wv, Wo)
    dev_w = jax.device_put(list(w4), [rt["sh"]] * 4)
    xhi, xnib = _host_xsT(query, key, value)
    dev_x = jax.device_put([xhi, xnib], [rt["sh"]] * 2)
    by_name = {"whqhi": dev_w[0], "whqnib": dev_w[1],
               "whohi": dev_w[2], "whonib": dev_w[3],
               "xhi": dev_x[0], "xnib": dev_x[1]}
    by_name.update(rt["dev_consts"])
    args = [by_name[n] for n in rt["in_names"]]
    outs = rt["fn"](*args, *rt["dev_zeros"])
    out_g = np.asarray(outs[0])                    # [8*(512+256), 1024] u8
    og = out_g.reshape(N_CORES, SLOC + SLOC // 2, DM)
    hi = og[:, 0:SLOC].reshape(-1, DM)
    nib = og[:, SLOC:].reshape(-1, 64, DM)
    bits = rt["post_bits"]
    v8 = bits.view(np.uint8).reshape(-1, 2 * DM)
    v8[:, 1::2] = hi                               # bits 8..15
    bb = bits.reshape(-1, 128, DM)
    nb = rt["post_nib"]
    np.bitwise_and(nib, np.uint8(0xF0), out=nb)
    bb.view(np.uint8)[:, 0:64, 0::2] = nb          # bits 4..7, rows 0:64
    np.left_shift(nib, np.uint8(4), out=nb)
    bb.view(np.uint8)[:, 64:128, 0::2] = nb        # bits 4..7, rows 64:128
    return bits.view(np.float16).reshape(B, S, DM).astype(np.float32)
